# revision 9
# baseline (speedup 1.0000x reference)
"""Distributed CustomLSTM kernel for 8 TRN2 NeuronCores.

Strategy: tensor-parallel over the 4*H gate dimension. Core r owns hidden
slice r (128 units) and the matching 4x128 gate columns (i|f|g|o). Each
timestep fuses x_t @ W_shard + h_t @ U_shard + bias into one 13-matmul
PSUM accumulation (bf16 operands, fp32 accumulate, h_t^T stationary).
After the elementwise LSTM cell update, h_t^T for the core's slice is
broadcast to the other 7 cores with XOR-relative remote SBUF DMAs so the
next step's matmul has the full hidden state. The physical core<->slot
permutation is probed once at runtime and baked into each core's U
row-tile order host-side, keeping the SPMD graph identical on all cores.
"""

import sys

sys.path.insert(0, "/opt/trn_rl_repo")

import numpy as np
import ml_dtypes

BF16 = ml_dtypes.bfloat16

N_CORES = 8
B = 64          # batch
INPUT = 512     # input feature size
HS = 1024       # hidden size
SLICE = HS // N_CORES   # 128 hidden units per core
GCOLS = 4 * SLICE       # 512 gate columns per core (i|f|g|o)
SEQ = 512
K_X = INPUT // 128      # 4 x K-tiles
K_H = HS // 128         # 8 h K-tiles
NXB = 4                 # x tile ring buffers

_CACHE = {}


def _mk_bass():
    import concourse.bacc as bacc
    return bacc.Bacc()


def _build_probe():
    """Tiny SPMD kernel: every core broadcasts its [128,8] f32 tile to slot k
    of peer (phys ^ k). Decoding the output reveals the core<->slot map."""
    import concourse.mybir as mybir

    nc = _mk_bass()
    P, S = 128, 8
    f32 = mybir.dt.float32
    inp = nc.declare_dram_parameter("inp", [P, S], f32, isOutput=False)
    out = nc.declare_dram_parameter("out", [P, N_CORES * S], f32, isOutput=True)
    with (
        nc.sbuf_tensor("src", [P, S], f32) as src,
        nc.sbuf_tensor("gth", [P, N_CORES * S], f32) as gth,
        nc.semaphore("dsem") as dsem,
        nc.semaphore("osem") as osem,
        nc.semaphore("psem") as psem,
        nc.semaphore("lsem") as lsem,
        nc.semaphore("rsem") as rsem,
        nc.semaphore("vsem") as vsem,
        nc.Block() as blk,
    ):
        @blk.gpsimd
        def _(g):
            g.dma_start(out=src[:, :], in_=inp[:, :]).then_inc(dsem, 16)
            g.wait_ge(dsem, 16)
            for k in range(1, N_CORES):
                rd = [None] * N_CORES
                rd[k] = (0, k)
                g.remote_dma_broadcast(
                    out_ap=gth[:, k * S:(k + 1) * S], in_ap=src[:, :],
                    remote_sem=rsem, local_sem=lsem, rdests=rd,
                ).then_inc(psem, 1)
            g.wait_ge(psem, N_CORES - 1)
            g.trigger_dma(count=N_CORES - 1)
            g.wait_ge(rsem, 2 * (N_CORES - 1))
            g.wait_ge(lsem, 16 * (N_CORES - 1))

        @blk.vector
        def _(v):
            v.wait_ge(dsem, 16)
            v.tensor_copy(gth[:, 0:S], src[:, :]).then_inc(vsem, 1)

        @blk.sync
        def _(s):
            s.wait_ge(vsem, 1)
            s.wait_ge(rsem, 2 * (N_CORES - 1))
            s.dma_start(out=out[:, :], in_=gth[:, :]).then_inc(osem, 16)
            s.wait_ge(osem, 16)

    nc.compile()
    return nc


def _probe_mapping():
    """Returns M[r][k] = source core whose tile lands in slot k on core r."""
    if "probe_map" in _CACHE:
        return _CACHE["probe_map"]
    from concourse.bass_utils import run_bass_kernel_spmd

    nc = _build_probe()
    P, S = 128, 8
    # unique constant per core
    in_maps = [
        {"inp": np.full((P, S), float(r + 1), dtype=np.float32)}
        for r in range(N_CORES)
    ]
    res = run_bass_kernel_spmd(nc, in_maps, core_ids=list(range(N_CORES)))
    M = np.full((N_CORES, N_CORES), -1, dtype=np.int64)
    for r in range(N_CORES):
        got = res.results[r]["out"]
        for k in range(N_CORES):
            v = got[0, k * S]
            s = int(round(float(v))) - 1
            assert 0 <= s < N_CORES, f"probe decode failed at core {r} slot {k}: {v}"
            M[r, k] = s
        assert M[r, 0] == r, f"slot 0 must be self on core {r}: {M[r]}"
        assert sorted(M[r].tolist()) == list(range(N_CORES)), f"core {r}: {M[r]}"
    _CACHE["probe_map"] = M
    return M


def _build_lstm(seq, reps=1):
    import concourse.mybir as mybir

    f32 = mybir.dt.float32
    bf = mybir.dt.bfloat16
    AF = mybir.ActivationFunctionType

    nc = _mk_bass()
    xT = nc.declare_dram_parameter("xT", [seq, 128, K_X * B], bf, isOutput=False)
    up = nc.declare_dram_parameter("Uperm", [128, K_H * GCOLS], bf, isOutput=False)
    w4 = nc.declare_dram_parameter("W4", [128, K_X * GCOLS], bf, isOutput=False)
    bi = nc.declare_dram_parameter("biasr", [1, GCOLS], bf, isOutput=False)
    on = nc.declare_dram_parameter("onesb", [1, B], bf, isOutput=False)
    idn = nc.declare_dram_parameter("ident", [B, B], f32, isOutput=False)
    hseq = nc.declare_dram_parameter("hseq", [seq, B, SLICE], f32, isOutput=True)
    clast = nc.declare_dram_parameter("clast", [B, SLICE], f32, isOutput=True)

    T = seq * reps
    from contextlib import ExitStack
    with ExitStack() as ctx:
        u_sb = ctx.enter_context(nc.sbuf_tensor("u_sb", [128, K_H * GCOLS], bf))
        w_sb = ctx.enter_context(nc.sbuf_tensor("w_sb", [128, K_X * GCOLS], bf))
        x_sb = ctx.enter_context(nc.sbuf_tensor("x_sb", [128, NXB * K_X * B], bf))
        g_sb = ctx.enter_context(nc.sbuf_tensor("g_sb", [128, 2 * N_CORES * B], bf))
        bias_sb = ctx.enter_context(nc.sbuf_tensor("bias_sb", [1, GCOLS], bf))
        ones_sb = ctx.enter_context(nc.sbuf_tensor("ones_sb", [1, B], bf))
        id_sb = ctx.enter_context(nc.sbuf_tensor("id_sb", [B, B], f32))
        gates_sb = ctx.enter_context(nc.sbuf_tensor("gates_sb", [B, 2 * GCOLS], f32))
        t1_sb = ctx.enter_context(nc.sbuf_tensor("t1_sb", [B, SLICE], f32))
        t2_sb = ctx.enter_context(nc.sbuf_tensor("t2_sb", [B, SLICE], f32))
        c_sb = ctx.enter_context(nc.sbuf_tensor("c_sb", [B, 2 * SLICE], f32))
        th_sb = ctx.enter_context(nc.sbuf_tensor("th_sb", [B, SLICE], f32))
        h_sb = ctx.enter_context(nc.sbuf_tensor("h_sb", [B, 2 * SLICE], f32))
        ps_g0 = ctx.enter_context(nc.psum_tensor([128, 512], f32))
        ps_g1 = ctx.enter_context(nc.psum_tensor([128, 512], f32))
        ps_t0 = ctx.enter_context(nc.psum_tensor([128, 512], f32))
        ps_t1 = ctx.enter_context(nc.psum_tensor([128, 512], f32))
        init_sem = ctx.enter_context(nc.semaphore("init_sem"))
        xdma = [ctx.enter_context(nc.semaphore(f"xdma{j}")) for j in range(NXB)]
        xuse = ctx.enter_context(nc.semaphore("xuse"))
        mm_done = ctx.enter_context(nc.semaphore("mm_done"))
        sig_done = ctx.enter_context(nc.semaphore("sig_done"))
        c_done = ctx.enter_context(nc.semaphore("c_done"))
        th_done = ctx.enter_context(nc.semaphore("th_done"))
        h_done = ctx.enter_context(nc.semaphore("h_done"))
        tp_done = ctx.enter_context(nc.semaphore("tp_done"))
        cast_done = ctx.enter_context(nc.semaphore("cast_done"))
        hdma = [ctx.enter_context(nc.semaphore(f"hdma{j}")) for j in range(2)]
        prep = ctx.enter_context(nc.semaphore("prep"))
        lsem = ctx.enter_context(nc.semaphore("lsem"))
        rsem = [[ctx.enter_context(nc.semaphore(f"rsem{k}_{par}"))
                 for par in range(2)] for k in range(1, N_CORES)]
        cdma = ctx.enter_context(nc.semaphore("cdma"))
        chain = ctx.enter_context(nc.semaphore("chain"))
        blk = ctx.enter_context(nc.Block())
        psg = [ps_g0.ap()[0:B, :], ps_g1.ap()[0:B, :]]
        pst = [ps_t0.ap()[:, 0:B], ps_t1.ap()[:, 0:B]]

        def gslot(par, k):  # gather slot k, parity par
            return g_sb[:, (par * N_CORES + k) * B:(par * N_CORES + k + 1) * B]

        def xslot(t, j):  # x K-tile j of step t
            base = (t % NXB) * K_X * B
            return x_sb[:, base + j * B: base + (j + 1) * B]

        def gate(par, i):  # gate chunk i of sigmoid output (0:i 1:f 2:g 3:o)
            return gates_sb[:, par * GCOLS + i * SLICE: par * GCOLS + (i + 1) * SLICE]

        def half(buf, par):  # [B, SLICE] half of a double-buffered tensor
            return buf[:, par * SLICE:(par + 1) * SLICE]

        @blk.tensor
        def _(te):
            te.wait_ge(init_sem, 80)
            te.wait_ge(xdma[0], 16)
            # open group(0): bias + x only (h_0 = 0)
            te.matmul(psg[0], ones_sb[:, :], bias_sb[:, :], start=True, stop=False)
            for j in range(K_X):
                mm = te.matmul(psg[0], xslot(0, j), w_sb[:, j * GCOLS:(j + 1) * GCOLS],
                               start=False, stop=(j == K_X - 1))
            mm.then_inc(mm_done, 1)
            te.nop().then_inc(xuse, 1)
            for t in range(T):
                p = t & 1
                if t >= 1:
                    # close group(t): 8 h-matmuls (slot 0 = own tile first)
                    te.wait_ge(cast_done, t)
                    te.matmul(psg[p], gslot(p, 0), u_sb[:, 0:GCOLS],
                              start=False, stop=False)
                    for k in range(1, K_H):
                        te.wait_ge(rsem[k - 1][(t - 1) & 1], 2 * ((t - 1) // 2 + 1))
                        mm = te.matmul(psg[p], gslot(p, k),
                                       u_sb[:, k * GCOLS:(k + 1) * GCOLS],
                                       start=False, stop=(k == K_H - 1))
                    mm.then_inc(mm_done, 1)
                if t + 1 < T:
                    # open group(t+1): bias + x (runs during step-t elementwise)
                    pn = (t + 1) & 1
                    te.wait_ge(sig_done, t)
                    te.matmul(psg[pn], ones_sb[:, :], bias_sb[:, :],
                              start=True, stop=False)
                    te.wait_ge(xdma[(t + 1) % NXB], 16 * ((t + 1) // NXB + 1))
                    for j in range(K_X):
                        mm = te.matmul(psg[pn], xslot(t + 1, j),
                                       w_sb[:, j * GCOLS:(j + 1) * GCOLS],
                                       start=False, stop=False)
                    mm.then_inc(xuse, 1)
                if t < T - 1:
                    # h^T for the broadcast
                    te.wait_ge(h_done, t + 1)
                    te.transpose(pst[p], half(h_sb, p), id_sb[:, :]
                                 ).then_inc(tp_done, 1)

        @blk.scalar
        def _(act):
            for t in range(T):
                p = t & 1
                act.wait_ge(mm_done, t + 1)
                act.activation(gates_sb[:, p * GCOLS:(p + 1) * GCOLS], psg[p],
                               AF.Sigmoid).then_inc(sig_done, 1)
                act.wait_ge(c_done, t + 2)
                act.wait_ge(h_done, t)
                act.activation(th_sb[:, :], half(c_sb, p), AF.Tanh
                               ).then_inc(th_done, 1)

        @blk.vector
        def _(dve):
            dve.memset(half(c_sb, 1), 0.0).then_inc(c_done, 1)  # c_{-1} = 0
            dve.wait_ge(c_done, 1)
            for t in range(T):
                p = t & 1
                q = 1 - p
                dve.wait_ge(sig_done, t + 1)
                dve.wait_ge(c_done, t + 1)
                dve.tensor_mul(t1_sb[:, :], gate(p, 0), gate(p, 2)
                               ).then_inc(chain, 1)   # i*g
                dve.tensor_mul(t2_sb[:, :], gate(p, 1), half(c_sb, q)
                               ).then_inc(chain, 1)   # f*c
                dve.wait_ge(chain, 2 * (t + 1))
                dve.tensor_add(half(c_sb, p), t1_sb[:, :], t2_sb[:, :]
                               ).then_inc(c_done, 1)  # c_done = t + 2 after step t
                dve.wait_ge(th_done, t + 1)
                if t >= 2:
                    dve.wait_ge(hdma[t & 1], 16 * (t // 2))
                dve.tensor_mul(half(h_sb, p), gate(p, 3), th_sb[:, :]
                               ).then_inc(h_done, 1)
                if t < T - 1:
                    dve.wait_ge(tp_done, t + 1)
                    if t >= 2:
                        dve.wait_ge(lsem, 112 * (t - 1))
                    dve.tensor_copy(gslot(q, 0), pst[p]).then_inc(cast_done, 1)

        @blk.gpsimd
        def _(g):
            for t in range(T - 1):
                q = 1 - (t & 1)
                # witness for the peer's slot-k buffer reuse: our own slot-k
                # arrivals (symmetric XOR pairing) prove the peer consumed
                # our previous send before we overwrite its gather slot
                if t >= 1:
                    for k in range(1, N_CORES):
                        g.wait_ge(rsem[k - 1][(t - 1) & 1], 2 * ((t - 1) // 2 + 1))
                for k in range(1, N_CORES):
                    rd = [None] * N_CORES
                    rd[k] = (0, k)
                    g.remote_dma_broadcast(
                        out_ap=gslot(q, k), in_ap=gslot(q, 0),
                        remote_sem=rsem[k - 1][t & 1], local_sem=lsem, rdests=rd,
                    ).then_inc(prep, 1)
                g.wait_ge(prep, 7 * (t + 1))
                g.wait_ge(cast_done, t + 1)
                if t >= 1:
                    g.wait_ge(lsem, 112 * t)
                g.trigger_dma(count=7)
            if T >= 2:
                g.wait_ge(lsem, 112 * (T - 1))

        @blk.sync
        def _(s):
            s.dma_start(out=u_sb[:, :], in_=up[:, :]).then_inc(init_sem, 16)
            s.dma_start(out=w_sb[:, :], in_=w4[:, :]).then_inc(init_sem, 16)
            s.dma_start(out=bias_sb[:, :], in_=bi[:, :]).then_inc(init_sem, 16)
            s.dma_start(out=ones_sb[:, :], in_=on[:, :]).then_inc(init_sem, 16)
            s.dma_start(out=id_sb[:, :], in_=idn[:, :]).then_inc(init_sem, 16)
            for m in range(min(3, T)):
                s.dma_start(out=x_sb[:, (m % NXB) * K_X * B:
                                     ((m % NXB) + 1) * K_X * B],
                            in_=xT[m % seq, :, :]).then_inc(xdma[m % NXB], 16)
            for t in range(T):
                p = t & 1
                m = t + 3
                if m < T:
                    s.wait_ge(xuse, t)
                    s.dma_start(out=x_sb[:, (m % NXB) * K_X * B:
                                         ((m % NXB) + 1) * K_X * B],
                                in_=xT[m % seq, :, :]).then_inc(xdma[m % NXB], 16)
                s.wait_ge(h_done, t + 1)
                s.dma_start(out=hseq[t % seq, :, :], in_=half(h_sb, p)
                            ).then_inc(hdma[p], 16)
            s.wait_ge(c_done, T + 1)
            s.dma_start(out=clast[:, :], in_=half(c_sb, (T - 1) & 1)
                        ).then_inc(cdma, 16)
            s.wait_ge(cdma, 16)
            s.wait_ge(hdma[0], 16 * ((T + 1) // 2))
            s.wait_ge(hdma[1], 16 * (T // 2))

    nc.compile()
    return nc


def _get_lstm(seq, reps=1):
    key = ("lstm", seq, reps)
    if key not in _CACHE:
        _CACHE[key] = _build_lstm(seq, reps)
    return _CACHE[key]


def _gate_cols(r):
    """Global columns of the 4H gate axis owned by core r (i|f|g|o order)."""
    return np.concatenate([np.arange(g * HS + r * SLICE, g * HS + (r + 1) * SLICE)
                           for g in range(4)])


def make_in_maps(x, W, U, bias, M, seq):
    """Per-core input dicts. M[r][k] = source core for gather slot k on core r."""
    x = np.asarray(x, dtype=np.float32)
    W = np.asarray(W, dtype=np.float32)
    U = np.asarray(U, dtype=np.float32)
    bias = np.asarray(bias, dtype=np.float32)

    # xT[t, p, j*B + b] = x[b, t, j*128 + p]  (shared by all cores)
    xT = x.transpose(1, 2, 0).reshape(seq, K_X, 128, B).transpose(0, 2, 1, 3)
    xT = np.ascontiguousarray(xT.reshape(seq, 128, K_X * B)).astype(BF16)

    ident = np.eye(B, dtype=np.float32)
    ones = np.ones((1, B), dtype=BF16)

    in_maps = []
    for r in range(N_CORES):
        cols = _gate_cols(r)
        Wr = W[:, cols]  # [512, 512]
        Ur = U[:, cols]  # [1024, 512]
        br = bias[cols]
        w4 = Wr.reshape(K_X, 128, GCOLS).transpose(1, 0, 2)
        w4 = np.ascontiguousarray(w4.reshape(128, K_X * GCOLS)).astype(BF16)
        # slot k multiplies h-slice of core M[r][k]
        uperm = np.stack([Ur[M[r][k] * SLICE:(M[r][k] + 1) * SLICE, :]
                          for k in range(K_H)], axis=1)  # [128, 8, 512]
        uperm = np.ascontiguousarray(uperm.reshape(128, K_H * GCOLS)).astype(BF16)
        in_maps.append({
            "xT": xT,
            "Uperm": uperm,
            "W4": w4,
            "biasr": br.reshape(1, GCOLS).astype(BF16),
            "onesb": ones,
            "ident": ident,
        })
    return in_maps


def assemble_output(results, seq):
    hs = np.concatenate([results[r]["hseq"] for r in range(N_CORES)], axis=-1)
    hidden_seq = np.ascontiguousarray(hs.transpose(1, 0, 2)).astype(np.float32)
    c_last = np.concatenate([results[r]["clast"] for r in range(N_CORES)],
                            axis=-1).astype(np.float32)
    h_last = np.ascontiguousarray(hidden_seq[:, -1, :])
    return hidden_seq, (h_last, c_last)


def run_lstm(x, W, U, bias, trace=False):
    from concourse.bass_utils import run_bass_kernel_spmd

    seq = x.shape[1]
    M = _probe_mapping()
    nc = _get_lstm(seq)
    in_maps = make_in_maps(x, W, U, bias, M, seq)
    res = run_bass_kernel_spmd(nc, in_maps, core_ids=list(range(N_CORES)),
                               trace=trace)
    out = assemble_output(res.results, seq)
    return out, res


def kernel(x, W, U, bias):
    out, _ = run_lstm(x, W, U, bias, trace=False)
    return out
